# revision 21
# baseline (speedup 1.0000x reference)
"""ASFormer layer (banded local attention + conv FFN) on 8 trn2 NeuronCores.

Sharding: sequence-parallel. (batch, seq-chunk) -> core: B=2 x 4 chunks of 512
tokens. Each core computes output tokens [start, start+512) of one batch,
reading a 768-token halo slice of x (banded attention needs +-64 keys, the
depthwise conv another +-1 token).

Layout strategy per core:
  - x token-major -> LN_a (bn_stats) -> PE-transpose -> a^T feature-major
  - QKV projection in feature-major (f32r matmuls, LN gain/bias folded into
    host-prepared weights via an appended ones-row K-tile)
  - attention computed as S^T[key, query] tiles (k-major) so softmax'd P^T
    feeds the ctx matmul directly with no transposes; softmax denominators
    via an all-ones stationary matmul; 1/denom via ACT ln->exp
  - w_o back to token-major, residual, LN_f/LN_c token-major, PE-transpose,
    depthwise conv as shifted DVE ops in feature-major, pwi (f32r) -> gelu
    (exact erf) -> pwo (bf16) -> token-major + residual -> out
"""

import numpy as np
import ml_dtypes

import concourse.bass as bass
import concourse.tile as tile
import concourse.mybir as mybir
from concourse.bass_utils import run_bass_kernel_spmd

F32 = mybir.dt.float32
F32R = mybir.dt.float32r
BF16 = mybir.dt.bfloat16
AF = mybir.ActivationFunctionType
ALU = mybir.AluOpType

B, S, D, H, HD, FF = 2, 2048, 512, 8, 64, 2048
WIN = 64
NCORES = 8
CHUNK = 512          # output tokens per core
EXT = 768            # x slice per core: tokens [start-128, start+640)
NQ = 640             # query grid: tokens [start-64, start+576)
NEG = -1e30
EPS = 1e-5

# window start (in ext coords) for each of the 6 key tiles
_WJ = [0, 64, 192, 320, 448, 512]
# ctx accumulation: for chunk c (queries ext [256c, 256c+256)), list of
# (j, lo, hi) with lo/hi in ext coords = intersection of window j with chunk
_CTX = {
    0: [(0, 0, 256), (1, 64, 256), (2, 192, 256)],
    1: [(1, 256, 320), (2, 256, 448), (3, 320, 512), (4, 448, 512)],
    2: [(3, 512, 576), (4, 512, 704), (5, 512, 768)],
}


def _fix_excess_waits(nc):
    """The pinned walrus rejects >1 sync wait on most instructions (>2 on
    EventSemaphore). Hoist excess waits onto wait-only EventSemaphore insts."""
    for f in nc.m.functions:
        for bb in f.blocks:
            insts = list(bb.instructions)
            out = []
            changed = False
            for inst in insts:
                si = inst.sync_info
                if si is not None and si.on_wait:
                    cap = 2 if isinstance(inst, mybir.InstEventSemaphore) else 1
                    waits = list(si.on_wait)
                    if len(waits) > cap:
                        extra = waits[cap:]
                        inst.sync_info = mybir.SyncInfo(
                            on_wait=waits[:cap], on_update=list(si.on_update))
                        k = 0
                        while extra:
                            chunk, extra = extra[:2], extra[2:]
                            out.append(mybir.InstEventSemaphore(
                                name=f"{inst.name}-waitsplit{k}",
                                engine=inst.engine, ins=[], outs=[],
                                sync_info=mybir.SyncInfo(on_wait=chunk, on_update=[]),
                            ))
                            k += 1
                        changed = True
                out.append(inst)
            if changed:
                bb.instructions[:] = out


def _build_program(cfg):
    """cfg = (apply_gfbf, apply_gcbc, apply_outmask, use_qkv_bias, use_wo_bias,
    use_pwi_bias, use_pwo_bias) booleans."""
    (apply_gfbf, apply_gcbc, apply_outmask,
     use_qkv_bias, use_wo_bias, use_pwi_bias, use_pwo_bias) = cfg
    nc = bass.Bass(target_bir_lowering=False, trn_type="TRN2")

    d_x = nc.dram_tensor("x_ext", [EXT, D], F32, kind="ExternalInput")
    d_wqkv = nc.dram_tensor("wqkv", [D + 1, 3 * D], F32R, kind="ExternalInput")
    d_wo = nc.dram_tensor("wo", [D + 1, D], BF16, kind="ExternalInput")
    d_wpwi = nc.dram_tensor("wpwi", [D + 1, FF], F32R, kind="ExternalInput")
    d_wpwo = nc.dram_tensor("wpwo", [FF + 1, D], BF16, kind="ExternalInput")
    d_convw = nc.dram_tensor("convw", [D, 3], F32, kind="ExternalInput")
    d_masks = nc.dram_tensor("masks", [6 * 128, 256], BF16, kind="ExternalInput")
    d_ident = nc.dram_tensor("ident", [128, 128], F32, kind="ExternalInput")
    d_ones = nc.dram_tensor("ones_r", [1, EXT], F32R, kind="ExternalInput")
    if apply_gfbf:
        d_gf = nc.dram_tensor("gf_b", [128, D], F32, kind="ExternalInput")
        d_bf = nc.dram_tensor("bf_b", [128, D], F32, kind="ExternalInput")
    if apply_gcbc:
        d_gc = nc.dram_tensor("gc_b", [128, D], F32, kind="ExternalInput")
        d_bc = nc.dram_tensor("bc_b", [128, D], F32, kind="ExternalInput")
        d_ppad = nc.dram_tensor("ppad", [NQ, 1], F32, kind="ExternalInput")
    if apply_outmask:
        d_om = nc.dram_tensor("outmask", [CHUNK, 1], F32, kind="ExternalInput")
    d_out = nc.dram_tensor("out", [CHUNK, D], F32, kind="ExternalOutput")

    import contextlib
    with tile.TileContext(nc) as tc, \
         tc.tile_pool(name="cst", bufs=1) as cst, \
         tc.tile_pool(name="pA", bufs=1) as pA:
        with contextlib.ExitStack() as stack:
            # ---- constants ----
            ident = cst.tile([128, 128], F32)
            ones_r = cst.tile([1, EXT], F32R)
            ones_bf64 = cst.tile([128, HD], BF16)
            nc.vector.memset(ones_bf64, 1.0)
            onesrow_bf = cst.tile([1, 128], BF16)
            nc.vector.memset(onesrow_bf, 1.0)
            eps_sb = cst.tile([128, 1], F32)
            nc.vector.memset(eps_sb, EPS)
            convw_sb = [cst.tile([128, 3], F32, tag=f"convw{i}", name=f"convw{i}") for i in range(4)]
            wo_sb = [cst.tile([64, D], BF16, tag=f"wo{h}", name=f"wo{h}") for h in range(H)]
            wo_bias = cst.tile([1, D], BF16)
            if apply_gfbf:
                gf_sb = cst.tile([128, D], F32)
                bf_sb = cst.tile([128, D], F32)
                nc.sync.dma_start(out=gf_sb, in_=d_gf[:, :])
                nc.sync.dma_start(out=bf_sb, in_=d_bf[:, :])
            if apply_gcbc:
                gc_sb = cst.tile([128, D], F32)
                bc_sb = cst.tile([128, D], F32)
                nc.sync.dma_start(out=gc_sb, in_=d_gc[:, :])
                nc.sync.dma_start(out=bc_sb, in_=d_bc[:, :])
                ppad_sb = [cst.tile([128, 1], F32, tag=f"ppad{t}", name=f"ppad{t}") for t in range(5)]
                for t in range(5):
                    nc.sync.dma_start(out=ppad_sb[t], in_=d_ppad[128 * t:128 * t + 128, :])
            if apply_outmask:
                om_sb = [cst.tile([128, 1], F32, tag=f"om{t}", name=f"om{t}") for t in range(4)]
                for t in range(4):
                    nc.sync.dma_start(out=om_sb[t], in_=d_om[128 * t:128 * t + 128, :])

            # ---- long-lived big tensors (span w_o .. end) ----
            wpwi_sb = [pA.tile([128, FF], F32R, tag=f"wpwi{i}", name=f"wpwi{i}") for i in range(4)]
            wpwi_bias = pA.tile([1, FF], F32R)
            wpwo_sb = [pA.tile([128, D], BF16, tag=f"wpwo{i}", name=f"wpwo{i}") for i in range(16)]
            wpwo_bias = pA.tile([1, D], BF16)
            x1 = [pA.tile([128, D], F32, tag=f"x1_{t}", name=f"x1_{t}") for t in range(5)]
            out_sb = [pA.tile([128, D], F32, tag=f"out{t}", name=f"out{t}") for t in range(4)]

            # ---- attention-era tensors (span qkv .. w_o) ----
            pB = stack.enter_context(tc.tile_pool(name="pB", bufs=1))
            qk_t = [pB.tile([128, EXT], F32R, tag=f"qk{i}", name=f"qk{i}") for i in range(8)]
            v_sb = [pB.tile([128, D], BF16, tag=f"v{i}", name=f"v{i}") for i in range(6)]
            ctxT = [pB.tile([64, EXT], BF16, tag=f"ctx{h}", name=f"ctx{h}") for h in range(H)]
            x_q = [pB.tile([128, D], F32, tag=f"xq{t}", name=f"xq{t}") for t in range(5)]
            masks_sb = pB.tile([128, 6 * 256], BF16, name="masks_sb")

            # ============ stage 1: LN_a + transpose ============
            with tc.tile_pool(name="pC", bufs=1) as pC, \
                 tc.tile_pool(name="pCw", bufs=2) as pCw, \
                 tc.tile_pool(name="psTR", bufs=2, space="PSUM") as psTR, \
                 tc.tile_pool(name="psQK", bufs=3, space="PSUM") as psQK:
                wqkv_sb = [pC.tile([128, 3 * D], F32R, tag=f"wqkv{i}", name=f"wqkv{i}") for i in range(4)]
                wqkv_bias = pC.tile([1, 3 * D], F32R)
                aT_all = pC.tile([128, 4 * EXT], F32R, name="aT_all")
                aT = [aT_all[:, EXT * i:EXT * (i + 1)] for i in range(4)]

                xts = []
                mv_all = pC.tile([128, 12], F32, name="mv_all")
                for t in range(6):
                    xt = pCw.tile([128, D], F32, tag="xt", name=f"xt{t}", bufs=6)
                    xts.append(xt)
                    nc.sync.dma_start(out=xt, in_=d_x[128 * t:128 * t + 128, :])
                    st = pCw.tile([128, 6], F32, tag="st")
                    nc.vector.bn_stats(out=st, in_=xt)
                    nc.vector.bn_aggr(out=mv_all[:, 2 * t:2 * t + 2], in_=st)
                # x is on chip; now stream in the weights needed next
                nc.sync.dma_start(out=ident, in_=d_ident[:, :])
                nc.sync.dma_start(out=ones_r, in_=d_ones[:, :])
                for i in range(4):
                    nc.sync.dma_start(out=wqkv_sb[i], in_=d_wqkv[128 * i:128 * i + 128, :])
                nc.sync.dma_start(out=wqkv_bias, in_=d_wqkv[D:D + 1, :])
                lnv_all = pC.tile([128, 6], F32, name="lnv_all")
                rstd_all = pC.tile([128, 6], F32, name="rstd_all")
                nc.scalar.activation(out=lnv_all,
                                     in_=mv_all.rearrange("p (t two) -> p t two", two=2)[:, :, 1:2],
                                     func=AF.Ln, bias=eps_sb, scale=1.0)
                nc.scalar.activation(out=rstd_all, in_=lnv_all, func=AF.Exp,
                                     bias=0.0, scale=-0.5)
                for t in range(6):
                    xt = xts[t]
                    ah = pCw.tile([128, D], F32, tag="ah")
                    nc.vector.tensor_scalar(out=ah, in0=xt, scalar1=mv_all[:, 2 * t:2 * t + 1],
                                            scalar2=rstd_all[:, t:t + 1], op0=ALU.subtract, op1=ALU.mult)
                    ptr = psTR.tile([128, 512], F32, tag="ptr")
                    for dd in range(4):
                        nc.tensor.matmul(ptr[:, 128 * dd:128 * dd + 128],
                                         ah[:, 128 * dd:128 * dd + 128], ident,
                                         is_transpose=True, start=(dd == 0),
                                         stop=(dd == 3), skip_group_check=True)
                    outv = aT_all.rearrange("p (g c) -> p g c", g=4)[:, :, 128 * t:128 * t + 128]
                    nc.scalar.copy(outv, ptr.rearrange("p (g c) -> p g c", g=4))

                # ============ stage 2+4 interleaved: V, then per f-tile pair qkv + 2 heads ============
                def emit_qkv_ft(ft):
                    for ch in range(2):
                        pq = psQK.tile([128, 384], F32, tag="pqv", name=f"pq_{ft}_{ch}")
                        for kt in range(4):
                            nc.tensor.matmul(pq, wqkv_sb[kt][:, 128 * ft:128 * ft + 128],
                                             aT[kt][:, 384 * ch:384 * ch + 384],
                                             start=(kt == 0),
                                             stop=(kt == 3 and not use_qkv_bias))
                        if use_qkv_bias:
                            nc.tensor.matmul(pq, wqkv_bias[:, 128 * ft:128 * ft + 128],
                                             ones_r[:, 384 * ch:384 * ch + 384],
                                             start=False, stop=True)
                        nc.scalar.copy(qk_t[ft][:, 384 * ch:384 * ch + 384], pq)

                def emit_v():
                    for tt in range(6):
                        pv = psQK.tile([128, D], F32, tag="pqv", name=f"pv_{tt}")
                        for kt in range(4):
                            nc.tensor.matmul(pv, aT[kt][:, 128 * tt:128 * tt + 128],
                                             wqkv_sb[kt][:, 2 * D:3 * D],
                                             start=(kt == 0),
                                             stop=(kt == 3 and not use_qkv_bias))
                        if use_qkv_bias:
                            nc.tensor.matmul(pv, ones_r[:, 128 * tt:128 * tt + 128],
                                             wqkv_bias[:, 2 * D:3 * D], start=False, stop=True)
                        nc.vector.tensor_copy(v_sb[tt], pv)

                def emit_head(h):
                    hp = 64 * (h % 2)
                    pTraw = pD.tile([128, 6 * 256], BF16, tag="pTraw", name=f"pTraw{h}")
                    for j in range(6):
                        pst = psST.tile([128, 256], F32, tag="pst", name=f"pst{h}_{j}")
                        nc.tensor.matmul(
                            pst,
                            qk_t[4 + h // 2][hp:hp + 64, 128 * j:128 * j + 128],
                            qk_t[h // 2][hp:hp + 64, _WJ[j]:_WJ[j] + 256],
                            start=True, stop=True)
                        nc.scalar.activation(out=pTraw[:, 256 * j:256 * j + 256],
                                             in_=pst, func=AF.Exp)
                    pT = pD.tile([128, 6 * 256], BF16, tag="pT", name=f"pT{h}")
                    nc.vector.tensor_mul(out=pT, in0=pTraw, in1=masks_sb)
                    tln = pD.tile([64, 768], F32, tag="tln", name=f"tln{h}", bufs=2)
                    trd = pD.tile([64, 768], F32, tag="trd", name=f"trd{h}", bufs=2)
                    pcxs = []
                    for c in range(3):
                        pcx = psCX.tile([64, 256], F32, tag="pcx", name=f"pcx{h}_{c}")
                        pdn = psDN.tile([64, 256], F32, tag="pdn", name=f"pdn{h}_{c}")
                        pcxs.append(pcx)
                        items = _CTX[c]
                        for idx, (j, lo, hi) in enumerate(items):
                            rhs = pT[:, 256 * j + lo - _WJ[j]:256 * j + hi - _WJ[j]]
                            first = idx == 0
                            last = idx == len(items) - 1
                            nc.tensor.matmul(pcx[:, lo - 256 * c:hi - 256 * c],
                                             v_sb[j][:, 64 * h:64 * h + 64], rhs,
                                             start=first, stop=last,
                                             skip_group_check=True)
                            nc.tensor.matmul(pdn[:, lo - 256 * c:hi - 256 * c],
                                             ones_bf64, rhs,
                                             start=first, stop=last,
                                             skip_group_check=True)
                        nc.scalar.activation(out=tln[:, 256 * c:256 * c + 256],
                                             in_=pdn, func=AF.Ln, bias=0.0, scale=1.0)
                    nc.scalar.activation(out=trd, in_=tln, func=AF.Exp,
                                         bias=0.0, scale=-1.0)
                    for c in range(3):
                        nc.vector.scalar_tensor_tensor(
                            out=ctxT[h][:, 256 * c:256 * c + 256],
                            in0=pcxs[c], scalar=1.0, in1=trd[:, 256 * c:256 * c + 256],
                            op0=ALU.mult, op1=ALU.mult)

            # ---- attention-phase + later loads ----
            nc.sync.dma_start(
                out=masks_sb.rearrange("p (j q) -> p j q", j=6),
                in_=d_masks.rearrange("(j p) q -> p j q", j=6))
            for h in range(H):
                nc.sync.dma_start(out=wo_sb[h], in_=d_wo[64 * h:64 * h + 64, :])
            nc.sync.dma_start(out=wo_bias, in_=d_wo[D:D + 1, :])
            for t in range(5):
                nc.sync.dma_start(out=x_q[t], in_=d_x[64 + 128 * t:192 + 128 * t, :])
            for i in range(4):
                nc.sync.dma_start(out=convw_sb[i], in_=d_convw[128 * i:128 * i + 128, :])

            # ---- FFN weights stream in during attention (scalar-engine HWDGE) ----
            for i in range(4):
                nc.scalar.dma_start(out=wpwi_sb[i], in_=d_wpwi[128 * i:128 * i + 128, :])
            nc.scalar.dma_start(out=wpwi_bias, in_=d_wpwi[D:D + 1, :])
            for i in range(16):
                nc.scalar.dma_start(out=wpwo_sb[i], in_=d_wpwo[128 * i:128 * i + 128, :])
            nc.scalar.dma_start(out=wpwo_bias, in_=d_wpwo[FF:FF + 1, :])

            # ============ emission: V, then (qkv pair, 2 heads) x4 ============
            with tc.tile_pool(name="pD", bufs=3) as pD_, \
                 tc.tile_pool(name="psST", bufs=3, space="PSUM") as psST_, \
                 tc.tile_pool(name="psCX", bufs=3, space="PSUM") as psCX_, \
                 tc.tile_pool(name="psDN", bufs=2, space="PSUM") as psDN_:
                globals().update()  # no-op
                pD, psST, psCX, psDN = pD_, psST_, psCX_, psDN_
                emit_v()
                for pair in range(4):
                    emit_qkv_ft(pair)
                    emit_qkv_ft(4 + pair)
                    emit_head(2 * pair)
                    emit_head(2 * pair + 1)

            # ============ stage 5: w_o + residual ============
            with tc.tile_pool(name="psAT", bufs=2, space="PSUM") as psAT:
                for tt in range(5):
                    pat = psAT.tile([128, D], F32, tag="pat")
                    for h in range(H):
                        nc.tensor.matmul(pat, ctxT[h][:, 64 + 128 * tt:192 + 128 * tt],
                                         wo_sb[h], start=(h == 0),
                                         stop=(h == H - 1 and not use_wo_bias))
                    if use_wo_bias:
                        nc.tensor.matmul(pat, onesrow_bf, wo_bias, start=False, stop=True)
                    nc.vector.scalar_tensor_tensor(out=x1[tt], in0=pat, scalar=1.0,
                                                   in1=x_q[tt], op0=ALU.mult, op1=ALU.add)

        # pools pB/pC/pD exited above via stack; continue in fresh scope
        with tc.tile_pool(name="pE", bufs=1) as pE, \
             tc.tile_pool(name="pEw", bufs=2) as pEw, \
             tc.tile_pool(name="psT2", bufs=2, space="PSUM") as psT2, \
             tc.tile_pool(name="psPI", bufs=3, space="PSUM") as psPI, \
             tc.tile_pool(name="psPO", bufs=2, space="PSUM") as psPO:
            yT_all = pE.tile([128, 4 * NQ], F32, name="yT_all")
            yT = [yT_all[:, NQ * i:NQ * (i + 1)] for i in range(4)]
            convT = [pE.tile([128, CHUNK], F32R, tag=f"cT{i}", name=f"cT{i}") for i in range(4)]
            g_sb = [pE.tile([128, CHUNK], BF16, tag=f"g{i}", name=f"g{i}") for i in range(16)]
            x1s = [pE.tile([128, D], F32, tag=f"x1s{i}", name=f"x1s{i}") for i in range(4)]

            # x1s = x1 shifted by 64 rows (SBUF->SBUF DMA moves across partitions)
            for t4 in range(4):
                nc.sync.dma_start(out=x1s[t4][0:64, :], in_=x1[t4][64:128, :])
                nc.sync.dma_start(out=x1s[t4][64:128, :], in_=x1[t4 + 1][0:64, :])

            # ---- LN_f / LN_c ----
            for tt in range(5):
                st1 = pEw.tile([128, 6], F32, tag="st1")
                mv1 = pEw.tile([128, 2], F32, tag="mv1")
                nc.vector.bn_stats(out=st1, in_=x1[tt])
                nc.vector.bn_aggr(out=mv1, in_=st1)
                l1 = pEw.tile([128, 1], F32, tag="l1")
                r1 = pEw.tile([128, 1], F32, tag="r1")
                nc.scalar.activation(out=l1, in_=mv1[:, 1:2], func=AF.Ln,
                                     bias=eps_sb, scale=1.0)
                nc.scalar.activation(out=r1, in_=l1, func=AF.Exp, bias=0.0, scale=-0.5)
                n1 = pEw.tile([128, D], F32, tag="n1")
                nc.vector.tensor_scalar(out=n1, in0=x1[tt], scalar1=mv1[:, 0:1],
                                        scalar2=r1, op0=ALU.subtract, op1=ALU.mult)
                if apply_gfbf:
                    y1a = pEw.tile([128, D], F32, tag="y1a")
                    nc.vector.tensor_mul(out=y1a, in0=n1, in1=gf_sb)
                    nc.vector.tensor_add(out=n1, in0=y1a, in1=bf_sb)
                st2 = pEw.tile([128, 6], F32, tag="st2")
                mv2 = pEw.tile([128, 2], F32, tag="mv2")
                nc.vector.bn_stats(out=st2, in_=n1)
                nc.vector.bn_aggr(out=mv2, in_=st2)
                l2 = pEw.tile([128, 1], F32, tag="l2")
                r2 = pEw.tile([128, 1], F32, tag="r2")
                nc.scalar.activation(out=l2, in_=mv2[:, 1:2], func=AF.Ln,
                                     bias=eps_sb, scale=1.0)
                nc.scalar.activation(out=r2, in_=l2, func=AF.Exp, bias=0.0, scale=-0.5)
                n2 = pEw.tile([128, D], F32, tag="n2")
                nc.vector.tensor_scalar(out=n2, in0=n1, scalar1=mv2[:, 0:1],
                                        scalar2=r2, op0=ALU.subtract, op1=ALU.mult)
                if apply_gcbc:
                    y2a = pEw.tile([128, D], F32, tag="y2a")
                    nc.vector.tensor_mul(out=y2a, in0=n2, in1=gc_sb)
                    nc.vector.tensor_add(out=n2, in0=y2a, in1=bc_sb)
                    nc.vector.tensor_scalar_mul(out=n2, in0=n2, scalar1=ppad_sb[tt])
                pt2 = psT2.tile([128, 512], F32, tag="pt2")
                for dd in range(4):
                    nc.tensor.matmul(pt2[:, 128 * dd:128 * dd + 128],
                                     n2[:, 128 * dd:128 * dd + 128], ident,
                                     is_transpose=True, start=(dd == 0),
                                     stop=(dd == 3), skip_group_check=True)
                outv = yT_all.rearrange("p (g c) -> p g c", g=4)[:, :, 128 * tt:128 * tt + 128]
                nc.scalar.copy(outv, pt2.rearrange("p (g c) -> p g c", g=4))

            # ---- depthwise conv (feature-major, shifted adds) ----
            for dd in range(4):
                c1 = pEw.tile([128, CHUNK], F32, tag="c1")
                nc.vector.tensor_scalar_mul(out=c1, in0=yT[dd][:, 65:65 + CHUNK],
                                            scalar1=convw_sb[dd][:, 2:3])
                c2 = pEw.tile([128, CHUNK], F32, tag="c2")
                nc.vector.scalar_tensor_tensor(out=c2, in0=yT[dd][:, 63:63 + CHUNK],
                                               scalar=convw_sb[dd][:, 0:1], in1=c1,
                                               op0=ALU.mult, op1=ALU.add)
                nc.vector.scalar_tensor_tensor(out=convT[dd], in0=yT[dd][:, 64:64 + CHUNK],
                                               scalar=convw_sb[dd][:, 1:2], in1=c2,
                                               op0=ALU.mult, op1=ALU.add)

            # ---- pwi + gelu ----
            for ffi in range(16):
                ppi = psPI.tile([128, CHUNK], F32, tag="ppi")
                for kt in range(4):
                    nc.tensor.matmul(ppi, wpwi_sb[kt][:, 128 * ffi:128 * ffi + 128],
                                     convT[kt], start=(kt == 0),
                                     stop=(kt == 3 and not use_pwi_bias))
                if use_pwi_bias:
                    nc.tensor.matmul(ppi, wpwi_bias[:, 128 * ffi:128 * ffi + 128],
                                     ones_r[:, 0:CHUNK], start=False, stop=True)
                nc.scalar.activation(out=g_sb[ffi], in_=ppi, func=AF.Gelu)

            # ---- pwo + final residual ----
            for t4 in range(4):
                ppo = psPO.tile([128, D], F32, tag="ppo")
                for ffi in range(16):
                    nc.tensor.matmul(ppo, g_sb[ffi][:, 128 * t4:128 * t4 + 128],
                                     wpwo_sb[ffi], start=(ffi == 0),
                                     stop=(ffi == 15 and not use_pwo_bias))
                if use_pwo_bias:
                    nc.tensor.matmul(ppo, onesrow_bf, wpwo_bias, start=False, stop=True)
                nc.vector.scalar_tensor_tensor(out=out_sb[t4], in0=ppo, scalar=1.0,
                                               in1=x1s[t4], op0=ALU.mult, op1=ALU.add)
                if apply_outmask:
                    nc.vector.tensor_scalar_mul(out=out_sb[t4], in0=out_sb[t4],
                                                scalar1=om_sb[t4])
                nc.sync.dma_start(out=d_out[128 * t4:128 * t4 + 128, :], in_=out_sb[t4])

    _fix_excess_waits(nc)
    return nc


_PROG_CACHE = {}


def _get_program(cfg):
    if cfg not in _PROG_CACHE:
        _PROG_CACHE[cfg] = _build_program(cfg)
    return _PROG_CACHE[cfg]


def _build_masks(key_mask_row, start):
    """Multiplicative {0,1} masks [6*128, 256] bf16 for one core (k-major S^T)."""
    out = np.zeros((6, 128, 256), np.float32)
    # key usability per ext position
    g_all = start - 128 + np.arange(EXT)
    k_ok = (g_all >= 0) & (g_all < S)
    k_ok &= key_mask_row[np.clip(g_all, 0, S - 1)]
    # a query is "live" if it is a real query position AND has >=1 usable
    # in-band key; otherwise it self-attends (finite junk, later zeroed --
    # matches the reference, whose all-masked rows are zeroed by the final
    # mask multiply before anything can observe them)
    q_live = np.zeros(EXT, bool)
    for e_q in range(64, 704):
        g_q = start - 128 + e_q
        if 0 <= g_q < S:
            lo, hi = max(0, e_q - WIN), min(EXT, e_q + WIN + 1)
            q_live[e_q] = k_ok[lo:hi].any()
    for j in range(6):
        kl = np.arange(128)
        ql = np.arange(256)
        e_k = 128 * j + kl[:, None]           # [128, 1]
        e_q = _WJ[j] + ql[None, :]            # [1, 256]
        band = np.abs(e_q - e_k) <= WIN
        ok = (q_live[e_q] & k_ok[e_k] & band) | ((~q_live[e_q]) & (e_k == e_q))
        out[j][np.broadcast_to(ok, (128, 256))] = 1.0
    return np.ascontiguousarray(out.reshape(6 * 128, 256).astype(ml_dtypes.bfloat16))


def prepare(**inputs):
    x = np.ascontiguousarray(np.asarray(inputs["x"], np.float32))
    key_mask = np.asarray(inputs["mask"]).astype(bool)
    ln_a_g = np.asarray(inputs["ln_a_g"], np.float32)
    ln_a_b = np.asarray(inputs["ln_a_b"], np.float32)
    w_qkv = np.asarray(inputs["w_qkv"], np.float32)
    b_qkv = np.asarray(inputs["b_qkv"], np.float32)
    w_o = np.asarray(inputs["w_o"], np.float32)
    b_o = np.asarray(inputs["b_o"], np.float32)
    ln_f_g = np.asarray(inputs["ln_f_g"], np.float32)
    ln_f_b = np.asarray(inputs["ln_f_b"], np.float32)
    ln_c_g = np.asarray(inputs["ln_c_g"], np.float32)
    ln_c_b = np.asarray(inputs["ln_c_b"], np.float32)
    dw_w = np.asarray(inputs["dw_w"], np.float32)
    dw_b = np.asarray(inputs["dw_b"], np.float32)
    pwi_w = np.asarray(inputs["pwi_w"], np.float32)
    pwi_b = np.asarray(inputs["pwi_b"], np.float32)
    pwo_w = np.asarray(inputs["pwo_w"], np.float32)
    pwo_b = np.asarray(inputs["pwo_b"], np.float32)

    apply_gfbf = not (np.all(ln_f_g == 1.0) and np.all(ln_f_b == 0.0))
    apply_gcbc = not np.all(ln_c_b == 0.0)
    apply_outmask = not key_mask.all()

    # ---- host weight prep ----
    scale = np.float32(1.0 / np.sqrt(HD))
    Wt = w_qkv.T.astype(np.float64)                      # [D, 3D]
    Wg = Wt * ln_a_g[:, None].astype(np.float64)
    brow = ln_a_b.astype(np.float64) @ Wt + b_qkv
    Wg[:, :D] *= scale
    brow[:D] *= scale
    wqkv_hat = np.ascontiguousarray(
        np.vstack([Wg, brow[None, :]]).astype(np.float32))

    wo_hat = np.ascontiguousarray(
        np.vstack([w_o.T, b_o[None, :]]).astype(ml_dtypes.bfloat16))

    if not apply_gcbc:
        convw_eff = dw_w[:, 0, :] * ln_c_g[:, None]
        cc = dw_b
    else:
        convw_eff = dw_w[:, 0, :]
        cc = dw_b
    convw_eff = np.ascontiguousarray(convw_eff.astype(np.float32))

    pwi_bias_row = pwi_b + pwi_w @ cc
    wpwi_hat = np.ascontiguousarray(
        np.vstack([pwi_w.T, pwi_bias_row[None, :]]).astype(np.float32))
    wpwo_hat = np.ascontiguousarray(
        np.vstack([pwo_w.T, pwo_b[None, :]]).astype(ml_dtypes.bfloat16))
    use_qkv_bias = bool(np.any(brow != 0.0))
    use_wo_bias = bool(np.any(b_o != 0.0))
    use_pwi_bias = bool(np.any(pwi_bias_row != 0.0))
    use_pwo_bias = bool(np.any(pwo_b != 0.0))
    cfg = (apply_gfbf, apply_gcbc, apply_outmask,
           use_qkv_bias, use_wo_bias, use_pwi_bias, use_pwo_bias)

    ident = np.eye(128, dtype=np.float32)
    ones_r = np.ones((1, EXT), np.float32)

    nc = _get_program(cfg)

    in_maps = []
    for core in range(NCORES):
        b, c = divmod(core, 4)
        start = CHUNK * c
        lo, hi = start - 128, start + 640
        x_ext = np.zeros((EXT, D), np.float32)
        s0, s1 = max(lo, 0), min(hi, S)
        x_ext[s0 - lo:s1 - lo] = x[b, s0:s1]
        m = {
            "x_ext": x_ext,
            "wqkv": wqkv_hat,
            "wo": wo_hat,
            "wpwi": wpwi_hat,
            "wpwo": wpwo_hat,
            "convw": convw_eff,
            "masks": _build_masks(key_mask[b], start),
            "ident": ident,
            "ones_r": ones_r,
        }
        if apply_gfbf:
            m["gf_b"] = np.ascontiguousarray(
                np.broadcast_to(ln_f_g[None, :], (128, D)).astype(np.float32))
            m["bf_b"] = np.ascontiguousarray(
                np.broadcast_to(ln_f_b[None, :], (128, D)).astype(np.float32))
        if apply_gcbc:
            m["gc_b"] = np.ascontiguousarray(
                np.broadcast_to(ln_c_g[None, :], (128, D)).astype(np.float32))
            m["bc_b"] = np.ascontiguousarray(
                np.broadcast_to(ln_c_b[None, :], (128, D)).astype(np.float32))
            gq = start - 64 + np.arange(NQ)
            m["ppad"] = ((gq >= 0) & (gq < S)).astype(np.float32)[:, None]
        if apply_outmask:
            m["outmask"] = key_mask[b, start:start + CHUNK].astype(np.float32)[:, None]
        in_maps.append(m)

    def assemble(per_core_outs):
        out = np.empty((B, S, D), np.float32)
        for core in range(NCORES):
            b, c = divmod(core, 4)
            out[b, CHUNK * c:CHUNK * (c + 1)] = per_core_outs[core]
        return out

    return nc, in_maps, assemble


def kernel(**inputs):
    nc, in_maps, assemble = prepare(**inputs)
    res = run_bass_kernel_spmd(nc, in_maps, core_ids=list(range(NCORES)))
    return assemble([res.results[core]["out"] for core in range(NCORES)])
